# revision 2
# baseline (speedup 1.0000x reference)
"""GroupedQueryAttention (B=2, S=2048, DIM=1024, 16 heads, 4 KV groups) on 8 trn2 cores.

v3. Sharding: core c -> (batch b = c // 4, kv-group g = c % 4); host sums the
4 group partials per batch and adds b_o.

Measured HW facts this design is built around (see mb_*.py):
- A 512-column matmul takes ~427ns when the board power-throttles the PE to
  1.2 GHz (happens on a majority of runs) and ~220ns otherwise; dtype is
  irrelevant to PE speed. So PE *instruction count* is the primary cost.
- fp8 DoubleRow processes 2 contraction-subtiles per instruction: projection
  chains with contraction >= 256 halve their instruction count.
- The Act engine's exp is ~1.1us per [128,1024] tile (dtype-independent) and
  must overlap the PE stream; es tiles are double-buffered and PV is emitted
  one m-step behind scores.
- Engines: only DVE/Act touch PSUM; XBAR DMA transpose (2-byte dtypes) does
  the xn transpose off-PE; x/weight DMAs ride the Act-engine queue so the
  sync queue (XBAR) never head-of-line blocks them.

Datapath: all-bf16 matmuls (fp8 anywhere on the main path costs ~2.5% rel
error — dot-product relative error does NOT average down with contraction
length — far over the 2e-2 budget half). LayerNorm stats alternate between
DVE (bn_stats) and Act (Copy/Square accum) per tile; x and weights load as
single-descriptor batched DMAs with x-quarter-0 first.
"""

import numpy as np

import concourse.bass as bass
import concourse.mybir as mybir
from concourse import bacc
from concourse.bass_utils import run_bass_kernel_spmd
from concourse.tile import TileContext
from concourse.masks import make_identity

B, S, DIM = 2, 2048, 1024
HEADS, DH, G = 16, 64, 4
HPG = HEADS // G              # 4 heads per group
EG = HPG * DH                 # 256 q columns per group
SCALE = DH ** -0.5
P = 128
NT_S = S // P                 # 16
NT_D = DIM // P               # 8
F32 = mybir.dt.float32
BF16 = mybir.dt.bfloat16
FP8 = mybir.dt.float8e4
AF = mybir.ActivationFunctionType
OP = mybir.AluOpType
AX = mybir.AxisListType
WO_SCALE = 1.0


def build_nc():
    nc = bacc.Bacc("TRN2", target_bir_lowering=False)
    x = nc.dram_tensor("x", [S, DIM], BF16, kind="ExternalInput")
    wq = nc.dram_tensor("wq", [DIM, EG], BF16, kind="ExternalInput")
    wkv = nc.dram_tensor("wkv", [DIM, 2 * DH], BF16, kind="ExternalInput")
    wo = nc.dram_tensor("wo", [EG, DIM], BF16, kind="ExternalInput")
    qb = nc.dram_tensor("qb", [2, P], F32, kind="ExternalInput")
    kvb = nc.dram_tensor("kvb", [1, P], F32, kind="ExternalInput")
    y = nc.dram_tensor("y", [DIM, S], BF16, kind="ExternalOutput")

    with TileContext(nc) as tc:
        with tc.tile_pool(name="persist", bufs=1) as pp:
            ident = pp.tile([P, P], F32)
            make_identity(nc, ident[:])
            identb = pp.tile([P, P], BF16)
            nc.vector.tensor_copy(out=identb[:], in_=ident[:])
            wq_sb = pp.tile([P, NT_D, EG], BF16)
            wkv_sb = pp.tile([P, NT_D, 2 * DH], BF16)
            wo_sb = pp.tile([P, 2, DIM], BF16)
            qb_sb = pp.tile([P, 2], F32)
            kvb_sb = pp.tile([P, 1], F32)
            eps_sb = pp.tile([P, 1], F32)
            nc.vector.memset(eps_sb[:], 1e-5)

            qT = pp.tile([P, 2, S], BF16)      # [e%128, e-chunk, s]
            kvT = pp.tile([P, S], BF16)        # rows 0:64 = kT, 64:128 = vT
            kdup = pp.tile([P, S], BF16)       # rows 64:128 = kT copy (odd heads)
            vones = pp.tile([P, NT_S, DH + 1], BF16)
            outT = pp.tile([P, 2, S], BF16)    # attention out
            xnT = pp.tile([P, NT_S, NT_D, P], BF16)  # [d%128, s-tile, d-chunk, s%128]

            # ---------- Phase 1: LayerNorm + XBAR transpose + projections ----
            with tc.tile_pool(name="ln", bufs=2) as lnp, \
                 tc.tile_pool(name="lnx", bufs=2) as lnxp, \
                 tc.tile_pool(name="lns", bufs=4) as lsp, \
                 tc.tile_pool(name="psP", bufs=3, space="PSUM") as ppp:
                # x quarter 0 first so LN starts immediately; then the batched
                # single-descriptor weight loads (all on the Act DMA queue).
                xqs = []
                xq0 = lnp.tile([P, 4, DIM], BF16, tag="xq")
                nc.scalar.dma_start(
                    out=xq0[:],
                    in_=x[0:512, :].rearrange("(t p) d -> p t d", p=P))
                xqs.append(xq0)
                nc.scalar.dma_start(
                    out=wq_sb[:],
                    in_=wq[:, :].rearrange("(c p) e -> p c e", p=P))
                nc.scalar.dma_start(
                    out=wkv_sb[:],
                    in_=wkv[:, :].rearrange("(c p) e -> p c e", p=P))
                nc.scalar.dma_start(
                    out=wo_sb[:],
                    in_=wo[:, :].rearrange("(c p) e -> p c e", p=P))
                for e in range(2):
                    nc.scalar.dma_start(out=qb_sb[:, e:e + 1], in_=qb[e, :, None])
                nc.scalar.dma_start(out=kvb_sb[:], in_=kvb[0, :, None])
                for quarter in range(4):
                    xq = xqs[quarter]
                    if quarter + 1 < 4:
                        nxq = lnp.tile([P, 4, DIM], BF16, tag="xq")
                        nc.scalar.dma_start(
                            out=nxq[:],
                            in_=x[(quarter + 1) * 512:(quarter + 2) * 512, :]
                                .rearrange("(t p) d -> p t d", p=P))
                        xqs.append(nxq)
                    for t in range(4):
                        i = quarter * 4 + t
                        xt = xq[:, t, :]
                        if t % 2 == 0:
                            # DVE path: bn_stats/bn_aggr
                            st6 = lsp.tile([P, 2, 6], F32, tag="st6")
                            nc.vector.bn_stats(st6[:, 0, :], xt[:, 0:512])
                            nc.vector.bn_stats(st6[:, 1, :], xt[:, 512:1024])
                            mv = lsp.tile([P, 2], F32, tag="mv")
                            nc.vector.bn_aggr(mv[:], st6[:])
                            mu = mv[:, 0:1]
                            var = mv[:, 1:2]
                        else:
                            # Act path: mean/meansq via activation accumulators
                            scr = lsp.tile([P, DIM], BF16, tag="scr")
                            mu_t = lsp.tile([P, 1], F32, tag="mu")
                            nc.scalar.activation(scr[:], xt[:], AF.Copy,
                                                 scale=1.0 / DIM, accum_out=mu_t[:])
                            msq = lsp.tile([P, 1], F32, tag="msq")
                            nc.scalar.activation(scr[:], xt[:], AF.Square,
                                                 scale=DIM ** -0.5,
                                                 accum_out=msq[:])
                            mu2 = lsp.tile([P, 1], F32, tag="mu2")
                            nc.vector.tensor_mul(mu2[:], mu_t[:], mu_t[:])
                            var_t = lsp.tile([P, 1], F32, tag="var")
                            nc.vector.tensor_sub(var_t[:], msq[:], mu2[:])
                            mu = mu_t[:]
                            var = var_t[:]
                        std = lsp.tile([P, 1], F32, tag="std")
                        nc.scalar.activation(std[:], var, AF.Sqrt, bias=eps_sb[:])
                        rstd = lsp.tile([P, 1], F32, tag="rstd")
                        nc.vector.reciprocal(rstd[:], std[:])
                        xn = lnp.tile([P, DIM], BF16, tag="xn")
                        nc.vector.tensor_scalar(
                            out=xn[:], in0=xt[:], scalar1=mu,
                            scalar2=rstd[:], op0=OP.subtract, op1=OP.mult)
                        nc.sync.dma_start_transpose(out=xnT[:, i, :, :], in_=xn[:])
                    # projections for this s-quarter
                    q0 = quarter * 512
                    for mc in range(2):
                        pq = ppp.tile([P, 512], F32, tag="pq")
                        for c in range(NT_D):
                            nc.tensor.matmul(
                                pq[:],
                                lhsT=wq_sb[:, c, mc * P:(mc + 1) * P],
                                rhs=xnT[:, quarter * 4:(quarter + 1) * 4, c, :],
                                start=(c == 0), stop=(c == NT_D - 1))
                        nc.vector.tensor_scalar_add(qT[:, mc, q0:q0 + 512], pq[:],
                                                    qb_sb[:, mc:mc + 1])
                    pkv = ppp.tile([P, 512], F32, tag="pq")
                    for c in range(NT_D):
                        nc.tensor.matmul(
                            pkv[:],
                            lhsT=wkv_sb[:, c, :],
                            rhs=xnT[:, quarter * 4:(quarter + 1) * 4, c, :],
                            start=(c == 0), stop=(c == NT_D - 1))
                    nc.vector.tensor_scalar_add(kvT[:, q0:q0 + 512], pkv[:], kvb_sb[:])

                # kT copy for odd heads; V natural [s, d] + ones column
                nc.sync.dma_start(out=kdup[64:128, :], in_=kvT[0:DH, :])
                nc.vector.memset(vones[:, :, DH], 1.0)
                for m in range(NT_S):
                    pv = ppp.tile([P, DH], BF16, tag="pv")
                    nc.tensor.transpose(pv[:], kvT[64:128, m * P:(m + 1) * P],
                                        identb[64:128, 64:128])
                    nc.vector.tensor_copy(out=vones[:, m, 0:DH], in_=pv[:])

            # ---------- Phase 2: attention ----------
            # PV is emitted one m-step behind scores so the PE queue does not
            # block on the exp; es tiles quad-buffered.
            with tc.tile_pool(name="att", bufs=4) as ap_, \
                 tc.tile_pool(name="bc", bufs=2) as bp, \
                 tc.tile_pool(name="psS", bufs=2, space="PSUM") as psp, \
                 tc.tile_pool(name="psO", bufs=2, space="PSUM") as pop:
                for h in range(HPG):
                    pr, ch = (h % 2) * DH, h // 2
                    kk = kvT[0:DH, :] if h % 2 == 0 else kdup[64:128, :]
                    po0 = pop.tile([DH + 1, 1024], F32, tag="po")
                    po1 = pop.tile([DH + 1, 1024], F32, tag="po")
                    pos = [po0, po1]
                    prev_es = None
                    for m in range(NT_S + 1):
                        cur_es = []
                        if m < NT_S:
                            for half in range(2):
                                q0 = half * 1024
                                ps = psp.tile([P, 1024], F32, tag="ps")
                                for n in range(2):
                                    nc.tensor.matmul(
                                        ps[:, n * 512:(n + 1) * 512],
                                        lhsT=kk[:, m * P:(m + 1) * P],
                                        rhs=qT[pr:pr + DH, ch,
                                               q0 + n * 512:q0 + (n + 1) * 512],
                                        start=True, stop=True)
                                es = ap_.tile([P, 1024], BF16, tag="es")
                                nc.scalar.activation(es[:], ps[:], AF.Exp,
                                                     scale=SCALE)
                                cur_es.append(es)
                        if m > 0:
                            for half in range(2):
                                for n in range(2):
                                    nc.tensor.matmul(
                                        pos[half][:, n * 512:(n + 1) * 512],
                                        lhsT=vones[:, m - 1, :],
                                        rhs=prev_es[half][:, n * 512:(n + 1) * 512],
                                        start=(m - 1 == 0), stop=(m - 1 == NT_S - 1))
                        prev_es = cur_es
                    for half in range(2):
                        q0 = half * 1024
                        po = pos[half]
                        # reciprocal straight from the PSUM denominator row so
                        # the gpsimd broadcast starts before the eviction copy
                        rc = bp.tile([1, 1024], F32, tag="rc")
                        nc.vector.reciprocal(rc[:], po[DH:DH + 1, :])
                        ot = bp.tile([DH, 1024], F32, tag="ot")
                        nc.vector.tensor_copy(out=ot[:], in_=po[0:DH, :])
                        rbs = bp.tile([DH, 1024], F32, tag="rbs")
                        nc.gpsimd.partition_broadcast(rbs[:], rc[:])
                        if h % 2 == 0:
                            nc.vector.tensor_mul(
                                outT[0:DH, ch, q0:q0 + 1024], ot[:], rbs[:])
                        else:
                            st = bp.tile([DH, 1024], BF16, tag="st")
                            nc.vector.tensor_mul(st[:], ot[:], rbs[:])
                            nc.sync.dma_start(
                                out=outT[DH:2 * DH, ch, q0:q0 + 1024], in_=st[:])

            # ---------- Phase 3: out-projection ----------
            with tc.tile_pool(name="yt", bufs=2) as yp, \
                 tc.tile_pool(name="psY", bufs=2, space="PSUM") as pyp:
                for mc in range(NT_D):
                    py = pyp.tile([P, S], F32, tag="py")
                    for ec in range(2):
                        for n in range(S // 512):
                            nc.tensor.matmul(
                                py[:, n * 512:(n + 1) * 512],
                                lhsT=wo_sb[:, ec, mc * P:(mc + 1) * P],
                                rhs=outT[:, ec, n * 512:(n + 1) * 512],
                                start=(ec == 0), stop=(ec == 1))
                    yt = yp.tile([P, S], BF16, tag="yt")
                    if mc % 2 == 0:
                        nc.scalar.activation(yt[:], py[:], AF.Copy)
                    else:
                        nc.vector.tensor_copy(out=yt[:], in_=py[:])
                    nc.sync.dma_start(out=y[mc * P:(mc + 1) * P, :], in_=yt[:])

    nc.compile()
    return nc


_NC = None


def _get_nc():
    global _NC
    if _NC is None:
        _NC = build_nc()
    return _NC


def make_in_maps(x, ln_gamma, ln_beta, w_q, w_k, w_v, w_o):
    bf = mybir.dt.np(BF16)
    f8 = mybir.dt.np(FP8)
    x = np.asarray(x, np.float32)
    g_ = np.asarray(ln_gamma, np.float32)
    b_ = np.asarray(ln_beta, np.float32)
    in_maps = []
    xb = [np.ascontiguousarray(x[b].astype(bf)) for b in range(B)]
    for core in range(8):
        b, g = divmod(core, 4)
        wq_s = np.ascontiguousarray(
            (g_[:, None] * w_q[:, g * EG:(g + 1) * EG]).astype(bf))
        wkv_s = np.ascontiguousarray(np.concatenate(
            [g_[:, None] * w_k[:, g * DH:(g + 1) * DH],
             g_[:, None] * w_v[:, g * DH:(g + 1) * DH]], axis=1).astype(bf))
        wo_s = np.ascontiguousarray(
            w_o[g * EG:(g + 1) * EG, :].astype(bf))
        qb_s = (b_ @ w_q[:, g * EG:(g + 1) * EG]).reshape(2, P).astype(np.float32)
        kvb_s = np.concatenate(
            [b_ @ w_k[:, g * DH:(g + 1) * DH],
             b_ @ w_v[:, g * DH:(g + 1) * DH]]).reshape(1, P).astype(np.float32)
        in_maps.append({
            "x": xb[b],
            "wq": wq_s, "wkv": wkv_s, "wo": wo_s,
            "qb": qb_s, "kvb": kvb_s,
        })
    return in_maps


def kernel(x, ln_gamma, ln_beta, w_q, w_k, w_v, w_o, b_o):
    nc = _get_nc()
    in_maps = make_in_maps(x, ln_gamma, ln_beta, w_q, w_k, w_v, w_o)
    res = run_bass_kernel_spmd(nc, in_maps, list(range(8)))
    out = np.zeros((B, S, DIM), np.float32)
    for core in range(8):
        b = core // 4
        out[b] += res.results[core]["y"].T
    out *= 1.0 / WO_SCALE
    out += np.asarray(b_o, np.float32)
    return out
